# revision 36
# baseline (speedup 1.0000x reference)
"""Trainium2 Bass kernel for a differentiable GRU decoder.

Per step t (max_len=32 steps), batch N=4096, E=512, V=1024:
    emb    = probs_{t-1} @ W_d2e.T            # [N, E]
    h      = GRUCell(emb, h)                  # [N, E]
    logits = h @ W_e2d.T + b_e2d              # [N, V]
    probs  = softmax(logits)                  # [N, V]  -> output[t]

Sharding: data-parallel over N across 8 cores (512 rows each), weights
replicated, the 32-step scan stays local per core — no collectives.

Design notes:
- Feature-major on-chip layout ([features on partitions, batch on free])
  lets every matmul chain without transposes; the per-core output is
  written feature-major as [T, V, 512] and un-transposed on the host
  during the gather.
- Matmul operands stream as bf16 by default (DEC_MM_DT=f32r switches to
  the fp32r path: ~2x slower, ~2e-4 output error vs ~4e-3 for bf16).
  PSUM accumulation is fp32 either way.  The GRU state keeps an fp32
  master for the elementwise update plus a rounded copy for the PE.
- Softmax row sums (a reduction over partitions) are computed with a
  ones-MATRIX matmul whose output lands pre-broadcast on all 128
  partitions (matmul cost scales only with the free dim, so M=128 costs
  the same as M=1); the reciprocal runs on the vector engine off the
  PE's critical path.  exp(logits) stays unnormalized: the 1/sum scale
  folds into the PSUM drain of the next step's emb matmul (per-batch
  scaling commutes with the contraction), and the fp32 output tiles get
  a full-precision normalize whose emission is deferred behind the next
  step's matmuls so it never blocks the PE.
- The recurrence's serial tail (last gate matmul -> h' -> logits) is
  kept short: n-gate biases are injected into PSUM by K=1 rank-1
  matmuls (stationary = bias row, moving = ones row) so no scalar-engine
  Identity pass sits on the critical path; (1-z) comes from a second
  sigmoid drain with scale=-1 and negated bias; z*h is computed early
  off-path; and the bf16 copy of h' that feeds the logits matmuls is
  written before the fp32 master.
- In each gate's PSUM accumulation the recurrent (W_hh @ h) half is
  emitted before the (W_ih @ emb) half, giving the scheduler
  emb-independent matmuls to run while the softmax reciprocal resolves.
"""

import os
import sys
import types

import numpy as np

import concourse.bacc as bacc
import concourse.mybir as mybir
import concourse.tile as tile

F32 = mybir.dt.float32
F32R = mybir.dt.float32r
BF16 = mybir.dt.bfloat16
AF = mybir.ActivationFunctionType

N_CORES = 8
MM_DT = F32R if os.environ.get("DEC_MM_DT", "bf16") == "f32r" else BF16
OUT_F32 = os.environ.get("DEC_OUT", "bf16") == "f32"


def _install_ntff_hook():
    """Register the axon NTFF profiling hook if the image's antenv lacks it."""
    try:
        import antenv.axon_hooks  # noqa: F401
        return
    except ImportError:
        pass
    try:
        from trn_agent_boot.trn_boot import _ntff_profile_via_ctypes

        hook = _ntff_profile_via_ctypes("/opt/axon/libaxon_pjrt.so")
    except Exception:
        hook = None
    mod = types.ModuleType("antenv.axon_hooks")
    mod.get_axon_ntff_profile_hook = lambda: hook
    mod.set_axon_ntff_profile_hook = lambda h: None
    sys.modules["antenv.axon_hooks"] = mod


_install_ntff_hook()


def _build(T, B, E, V):
    """Build the per-core Bacc module. B = per-core batch (free dim)."""
    KE = E // 128  # E-tiles (4)
    KV = V // 128  # V-tiles (8)

    nc = bacc.Bacc(None, target_bir_lowering=False)

    wdt = F32 if MM_DT == F32R else BF16  # dram dtype for weight streams
    xT = nc.dram_tensor("xT", [E, B], F32, kind="ExternalInput")
    wd2eT = nc.dram_tensor("wd2eT", [V, E], wdt, kind="ExternalInput")
    wihT = nc.dram_tensor("wihT", [E, 3 * E], wdt, kind="ExternalInput")
    whhT = nc.dram_tensor("whhT", [E, 3 * E], wdt, kind="ExternalInput")
    we2dT = nc.dram_tensor("we2dT", [E, V], wdt, kind="ExternalInput")
    # b_rz: cols [0:2*KE) = (b_ih+b_hh) for r,z; cols [2*KE:3*KE) = negated z part
    brz = nc.dram_tensor("brz", [128, 3 * KE], F32, kind="ExternalInput")
    bihn = nc.dram_tensor("bihn", [128, KE], F32, kind="ExternalInput")
    bhhn = nc.dram_tensor("bhhn", [128, KE], F32, kind="ExternalInput")
    be2d = nc.dram_tensor("be2d", [128, KV], F32, kind="ExternalInput")
    # unnormalized exp(logits) + per-step 1/rowsum; the host normalizes
    # during the gather (identical arithmetic to an on-device multiply,
    # since the on-device product would read the same rounded exp tiles)
    edt = BF16 if (MM_DT == BF16 and not OUT_F32) else F32
    out_e = nc.dram_tensor("out_e", [T, V, B], edt, kind="ExternalOutput")
    out_r = nc.dram_tensor("out_r", [T, 1, B], F32, kind="ExternalOutput")

    with tile.TileContext(nc) as tc:
        with (
            tc.tile_pool(name="w", bufs=1) as wp,
            tc.tile_pool(name="sb", bufs=1) as sb,
            tc.tile_pool(name="ps", bufs=1, space="PSUM") as pp,
        ):
            # ---- persistent weights, in first-use order (w_hh feeds t=0) ----
            def load_w(name, dram_ap, cols):
                if MM_DT == BF16:
                    wt = wp.tile([128, cols], BF16, name=name, tag=name)
                    nc.sync.dma_start(wt[:], dram_ap)
                else:
                    st = sb.tile([128, cols], F32, name="stage", tag="stage", bufs=2)
                    nc.sync.dma_start(st[:], dram_ap)
                    wt = wp.tile([128, cols], F32R, name=name, tag=name)
                    nc.vector.tensor_copy(wt[:], st[:])
                return wt

            # initial state h = x first (the first gh matmul needs it)
            hT = []  # fp32 master
            hM = []  # MM_DT matmul copy
            for m in range(KE):
                hf = sb.tile([128, B], F32, name="h", tag="h", bufs=8)
                nc.sync.dma_start(hf[:], xT[m * 128 : (m + 1) * 128, :])
                hT.append(hf)
                hm = sb.tile([128, B], MM_DT, name="hmm", tag="hmm", bufs=8)
                nc.scalar.copy(hm[:], hf[:])
                hM.append(hm)

            w_hh = [
                load_w(f"w_hh{k}", whhT[k * 128 : (k + 1) * 128, :], 3 * E)
                for k in range(KE)
            ]
            w_e2d = [
                load_w(f"w_e2d{k}", we2dT[k * 128 : (k + 1) * 128, :], V)
                for k in range(KE)
            ]
            w_d2e = [
                load_w(f"w_d2e{k}", wd2eT[k * 128 : (k + 1) * 128, :], E)
                for k in range(KV)
            ]
            w_ih = [
                load_w(f"w_ih{k}", wihT[k * 128 : (k + 1) * 128, :], 3 * E)
                for k in range(KE)
            ]

            b_rz = wp.tile([128, 3 * KE], F32, name="b_rz", tag="b_rz")
            nc.sync.dma_start(b_rz[:], brz[:])
            b_e2d = wp.tile([128, KV], F32, name="b_e2d", tag="b_e2d")
            nc.sync.dma_start(b_e2d[:], be2d[:])
            b_ihn = wp.tile([128, KE], F32, name="b_ihn", tag="b_ihn")
            nc.sync.dma_start(b_ihn[:], bihn[:])
            b_hhn = wp.tile([128, KE], F32, name="b_hhn", tag="b_hhn")
            nc.sync.dma_start(b_hhn[:], bhhn[:])

            ones_f32 = wp.tile([128, 128], F32, name="ones_f32", tag="ones_f32")
            nc.gpsimd.memset(ones_f32[:], 1.0)
            ones_mat = wp.tile([128, 128], MM_DT, name="ones_mat", tag="ones_mat")
            nc.vector.tensor_copy(ones_mat[:], ones_f32[:])

            eT = None  # unnormalized exp(logits) of previous step (MM_DT)
            rbc = None  # 1/rowsum broadcast [128, B] fp32

            for t in range(T):
                # ---- emb = softmax_{t-1} @ W_d2e.T (feature-major [E, B]);
                # normalization folded into the PSUM drain ----
                embT = None
                if t > 0:
                    H = B // 2
                    embT = []
                    for m in range(KE):
                        ps = pp.tile([128, B], F32, name="ps_mm", tag="mm", bufs=7)
                        for k in range(KV):
                            nc.tensor.matmul(
                                ps[:],
                                w_d2e[k][:, m * 128 : (m + 1) * 128],
                                eT[k][:],
                                start=(k == 0),
                                stop=(k == KV - 1),
                            )
                        ev = sb.tile([128, B], MM_DT, name="embT", tag="embT", bufs=8)
                        # drain in halves so each half only waits on its
                        # half of the reciprocal
                        nc.vector.tensor_mul(ev[:, :H], ps[:, :H], rbc[:, :H])
                        nc.vector.tensor_mul(ev[:, H:], ps[:, H:], rbc[:, H:])
                        embT.append(ev)

                # ---- gates r, z: sigmoid(gh + gx + biases); gh emitted first
                # so the PE has emb-independent work during the softmax tail.
                # z additionally drains (1-z) via sigmoid(-x) and z*h early ----
                r_g = []
                z_g = []  # z * h_old
                omz_g = []  # 1 - z
                for g in range(2):
                    # all four gh accumulation halves first (h-dependent only),
                    # then the four gx halves (emb-dependent): Bacc hoists each
                    # matmul group's waits onto the most recent LDWEIGHTS, so
                    # this keeps ~16 emb-independent matmuls in front of the
                    # softmax-reciprocal dependency
                    ps_g = []
                    for m in range(KE):
                        col = g * E + m * 128
                        ps = pp.tile([128, B], F32, name="ps_mm", tag="mm", bufs=7)
                        for k in range(KE):
                            nc.tensor.matmul(
                                ps[:],
                                w_hh[k][:, col : col + 128],
                                hM[k][:],
                                start=(k == 0),
                                stop=(t == 0 and k == KE - 1),
                            )
                        ps_g.append(ps)
                    if t > 0:
                        for m in range(KE):
                            col = g * E + m * 128
                            for k in range(KE):
                                nc.tensor.matmul(
                                    ps_g[m][:],
                                    w_ih[k][:, col : col + 128],
                                    embT[k][:],
                                    start=False,
                                    stop=(k == KE - 1),
                                )
                    for m in range(KE):
                        ps = ps_g[m]
                        j = g * KE + m
                        if g == 0:
                            gt = sb.tile(
                                [128, B], F32, name="gate_r", tag="gate_r", bufs=4
                            )
                            nc.scalar.activation(
                                gt[:], ps[:], AF.Sigmoid, bias=b_rz[:, j : j + 1]
                            )
                            r_g.append(gt)
                        else:
                            zt = sb.tile(
                                [128, B], F32, name="gate_z", tag="gate_z", bufs=4
                            )
                            nc.scalar.activation(
                                zt[:], ps[:], AF.Sigmoid, bias=b_rz[:, j : j + 1]
                            )
                            oz = sb.tile(
                                [128, B], F32, name="gate_omz", tag="gate_omz", bufs=4
                            )
                            nj = 2 * KE + m
                            nc.scalar.activation(
                                oz[:],
                                ps[:],
                                AF.Sigmoid,
                                bias=b_rz[:, nj : nj + 1],
                                scale=-1.0,
                            )
                            omz_g.append(oz)
                            zh = sb.tile(
                                [128, B], F32, name="zh", tag="zh", bufs=4
                            )
                            nc.gpsimd.tensor_mul(zh[:], zt[:], hT[m][:])
                            z_g.append(zh)

                # ---- n gate: tanh(xn + b_ihn + r * (hn + b_hhn)); b_hhn lands
                # via an off-critical-path Identity drain, b_ihn rides the
                # Tanh's per-partition bias ----
                hnb_g = []
                for m in range(KE):
                    col = 2 * E + m * 128
                    ps = pp.tile([128, B], F32, name="ps_mm", tag="mm", bufs=7)
                    for k in range(KE):
                        nc.tensor.matmul(
                            ps[:],
                            w_hh[k][:, col : col + 128],
                            hM[k][:],
                            start=(k == 0),
                            stop=(k == KE - 1),
                        )
                    hv = sb.tile([128, B], F32, name="hnb", tag="hnb", bufs=4)
                    nc.scalar.activation(
                        hv[:], ps[:], AF.Identity, bias=b_hhn[:, m : m + 1]
                    )
                    hnb_g.append(hv)

                ps_xn = None
                if t > 0:
                    ps_xn = []
                    for m in range(KE):
                        col = 2 * E + m * 128
                        ps = pp.tile([128, B], F32, name="ps_mm", tag="mm", bufs=7)
                        for k in range(KE):
                            nc.tensor.matmul(
                                ps[:],
                                w_ih[k][:, col : col + 128],
                                embT[k][:],
                                start=(k == 0),
                                stop=(k == KE - 1),
                            )
                        ps_xn.append(ps)

                # ---- h' = (1-z)*n + z*h, bf16 matmul copy written first ----
                hN = []
                hNM = []
                for m in range(KE):
                    t2 = sb.tile([128, B], F32, name="t2", tag="t2", bufs=4)
                    nc.vector.tensor_mul(t2[:], r_g[m][:], hnb_g[m][:])
                    if t > 0:
                        nc.vector.tensor_add(t2[:], t2[:], ps_xn[m][:])
                    nc.scalar.activation(
                        t2[:], t2[:], AF.Tanh, bias=b_ihn[:, m : m + 1]
                    )  # n, in place
                    nc.vector.tensor_mul(t2[:], t2[:], omz_g[m][:])  # (1-z)*n
                    hm = sb.tile([128, B], MM_DT, name="hmm", tag="hmm", bufs=8)
                    nc.vector.tensor_add(hm[:], t2[:], z_g[m][:])
                    hNM.append(hm)
                    hf = sb.tile([128, B], F32, name="h", tag="h", bufs=8)
                    nc.vector.tensor_add(hf[:], t2[:], z_g[m][:])
                    hN.append(hf)
                hT = hN
                hM = hNM

                # ---- logits = h' @ W_e2d.T + b_e2d; eT = exp(logits); the
                # row-sum matmuls (reduction over V partitions, pre-broadcast
                # via a ones matrix) are interleaved so the reciprocal can
                # start as soon as the last exp lands ----
                eT = []
                ps_s = pp.tile([128, B], F32, name="ps_s", tag="srow", bufs=1)
                for j in range(KV):
                    ps = pp.tile([128, B], F32, name="ps_mm", tag="mm", bufs=7)
                    for k in range(KE):
                        nc.tensor.matmul(
                            ps[:],
                            w_e2d[k][:, j * 128 : (j + 1) * 128],
                            hM[k][:],
                            start=(k == 0),
                            stop=(k == KE - 1),
                        )
                    if OUT_F32 and MM_DT == BF16:
                        # fp32 exp for the output stream; rounded copy for
                        # the PE (removes the bf16 rounding from the output)
                        ef = sb.tile([128, B], F32, name="eTf", tag="eTf", bufs=4)
                        nc.scalar.activation(
                            ef[:], ps[:], AF.Exp, bias=b_e2d[:, j : j + 1]
                        )
                        nc.sync.dma_start(
                            out_e[t, j * 128 : (j + 1) * 128, :], ef[:]
                        )
                        ev = sb.tile([128, B], MM_DT, name="eT", tag="eT", bufs=12)
                        nc.vector.tensor_copy(ev[:], ef[:])
                    else:
                        ev = sb.tile([128, B], MM_DT, name="eT", tag="eT", bufs=12)
                        nc.scalar.activation(
                            ev[:], ps[:], AF.Exp, bias=b_e2d[:, j : j + 1]
                        )
                        if MM_DT == BF16:
                            nc.sync.dma_start(
                                out_e[t, j * 128 : (j + 1) * 128, :], ev[:]
                            )
                        else:
                            nc.sync.dma_start(
                                out_e[t, j * 128 : (j + 1) * 128, :],
                                ev[:].bitcast(F32),
                            )
                    eT.append(ev)
                for j in range(KV):
                    nc.tensor.matmul(
                        ps_s[:],
                        ones_mat[:],
                        eT[j][:],
                        start=(j == 0),
                        stop=(j == KV - 1),
                    )
                H = B // 2
                rbc = sb.tile([128, B], F32, name="rbc", tag="rbc", bufs=2)
                nc.vector.reciprocal(rbc[:, :H], ps_s[:, :H])
                nc.vector.reciprocal(rbc[:, H:], ps_s[:, H:])
                nc.sync.dma_start(out_r[t, :, :], rbc[0:1, :])

    nc.compile()
    return nc


def _prep_inputs(x, W_d2e, W_ih, W_hh, b_ih, b_hh, W_e2d, b_e2d):
    E = x.shape[1]
    V = np.asarray(W_e2d).shape[0]
    KE = E // 128
    KV = V // 128

    if MM_DT == BF16:
        import ml_dtypes

        wnp = ml_dtypes.bfloat16
    else:
        wnp = np.float32

    def c(a, dt=np.float32):
        return np.ascontiguousarray(np.asarray(a, dtype=np.float32).astype(dt))

    b_ih = np.asarray(b_ih, dtype=np.float32)
    b_hh = np.asarray(b_hh, dtype=np.float32)
    brz_sum = (b_ih + b_hh)[: 2 * E].reshape(2 * KE, 128).T  # [128, 8]
    brz_negz = -(b_ih + b_hh)[E : 2 * E].reshape(KE, 128).T  # [128, 4]

    shared = {
        "wd2eT": c(np.asarray(W_d2e).T, wnp),  # [V, E]
        "wihT": c(np.asarray(W_ih).T, wnp),  # [E, 3E]
        "whhT": c(np.asarray(W_hh).T, wnp),
        "we2dT": c(np.asarray(W_e2d).T, wnp),  # [E, V]
        "brz": c(np.concatenate([brz_sum, brz_negz], axis=1)),  # [128, 12]
        "bihn": c(b_ih[2 * E :].reshape(KE, 128).T),
        "bhhn": c(b_hh[2 * E :].reshape(KE, 128).T),
        "be2d": c(np.asarray(b_e2d).reshape(KV, 128).T),
    }
    N = x.shape[0]
    B = N // N_CORES
    in_maps = []
    for core in range(N_CORES):
        m = dict(shared)
        m["xT"] = c(np.asarray(x)[core * B : (core + 1) * B, :].T)  # [E, B]
        in_maps.append(m)
    return in_maps, B


def _run(inputs, trace=False):
    from concourse.bass_utils import run_bass_kernel_spmd

    x = np.asarray(inputs["x"], dtype=np.float32)
    T = int(inputs["max_len"])
    N, E = x.shape
    V = np.asarray(inputs["W_e2d"]).shape[0]
    assert N % N_CORES == 0 and E % 128 == 0 and V % 128 == 0

    in_maps, B = _prep_inputs(
        x,
        inputs["W_d2e"],
        inputs["W_ih"],
        inputs["W_hh"],
        inputs["b_ih"],
        inputs["b_hh"],
        inputs["W_e2d"],
        inputs["b_e2d"],
    )
    nc = _build(T, B, E, V)
    res = run_bass_kernel_spmd(
        nc, in_maps, core_ids=list(range(N_CORES)), trace=trace
    )

    full = np.empty((T, N, V), dtype=np.float32)
    for core in range(N_CORES):
        e = np.asarray(res.results[core]["out_e"], dtype=np.float32)  # [T, V, B]
        rinv = np.asarray(res.results[core]["out_r"], dtype=np.float32)  # [T, 1, B]
        full[:, core * B : (core + 1) * B, :] = np.transpose(e * rinv, (0, 2, 1))
    return full, res


def kernel(**inputs):
    full, _ = _run(inputs, trace=False)
    return full


def run_traced(**inputs):
    return _run(inputs, trace=True)


# revision 39
# speedup vs baseline: 1.0485x; 1.0485x over previous
"""Trainium2 Bass kernel for a differentiable GRU decoder.

Per step t (max_len=32 steps), batch N=4096, E=512, V=1024:
    emb    = probs_{t-1} @ W_d2e.T            # [N, E]
    h      = GRUCell(emb, h)                  # [N, E]
    logits = h @ W_e2d.T + b_e2d              # [N, V]
    probs  = softmax(logits)                  # [N, V]  -> output[t]

Sharding: data-parallel over N across 8 cores (512 rows each), weights
replicated, the 32-step scan stays local per core — no collectives.

Design notes:
- Feature-major on-chip layout ([features on partitions, batch on free])
  lets every matmul chain without transposes; the per-core output is
  written feature-major as [T, V, 512] and un-transposed on the host
  during the gather.
- Matmul operands stream as bf16 by default (DEC_MM_DT=f32r switches to
  the fp32r path: ~2x slower, ~2e-4 output error vs ~4e-3 for bf16).
  PSUM accumulation is fp32 either way.  The GRU state keeps an fp32
  master for the elementwise update plus a rounded copy for the PE.
- Softmax row sums (a reduction over partitions) are computed with a
  ones-MATRIX matmul whose output lands pre-broadcast on all 128
  partitions (matmul cost scales only with the free dim, so M=128 costs
  the same as M=1); the reciprocal runs on the vector engine off the
  PE's critical path.  exp(logits) stays unnormalized: the 1/sum scale
  folds into the PSUM drain of the next step's emb matmul (per-batch
  scaling commutes with the contraction), and the fp32 output tiles get
  a full-precision normalize whose emission is deferred behind the next
  step's matmuls so it never blocks the PE.
- The recurrence's serial tail (last gate matmul -> h' -> logits) is
  kept short: n-gate biases are injected into PSUM by K=1 rank-1
  matmuls (stationary = bias row, moving = ones row) so no scalar-engine
  Identity pass sits on the critical path; (1-z) comes from a second
  sigmoid drain with scale=-1 and negated bias; z*h is computed early
  off-path; and the bf16 copy of h' that feeds the logits matmuls is
  written before the fp32 master.
- In each gate's PSUM accumulation the recurrent (W_hh @ h) half is
  emitted before the (W_ih @ emb) half, giving the scheduler
  emb-independent matmuls to run while the softmax reciprocal resolves.
"""

import os
import sys
import types

import numpy as np

import concourse.bacc as bacc
import concourse.mybir as mybir
import concourse.tile as tile

F32 = mybir.dt.float32
F32R = mybir.dt.float32r
BF16 = mybir.dt.bfloat16
AF = mybir.ActivationFunctionType

N_CORES = 8
MM_DT = F32R if os.environ.get("DEC_MM_DT", "bf16") == "f32r" else BF16
OUT_F32 = os.environ.get("DEC_OUT", "bf16") == "f32"


def _install_ntff_hook():
    """Register the axon NTFF profiling hook if the image's antenv lacks it."""
    try:
        import antenv.axon_hooks  # noqa: F401
        return
    except ImportError:
        pass
    try:
        from trn_agent_boot.trn_boot import _ntff_profile_via_ctypes

        hook = _ntff_profile_via_ctypes("/opt/axon/libaxon_pjrt.so")
    except Exception:
        hook = None
    mod = types.ModuleType("antenv.axon_hooks")
    mod.get_axon_ntff_profile_hook = lambda: hook
    mod.set_axon_ntff_profile_hook = lambda h: None
    sys.modules["antenv.axon_hooks"] = mod


_install_ntff_hook()


def _build(T, B, E, V):
    """Build the per-core Bacc module. B = per-core batch (free dim)."""
    KE = E // 128  # E-tiles (4)
    KV = V // 128  # V-tiles (8)

    nc = bacc.Bacc(None, target_bir_lowering=False)

    wdt = F32 if MM_DT == F32R else BF16  # dram dtype for weight streams
    xT = nc.dram_tensor("xT", [E, B], F32, kind="ExternalInput")
    wd2eT = nc.dram_tensor("wd2eT", [V, E], wdt, kind="ExternalInput")
    wihT = nc.dram_tensor("wihT", [E, 3 * E], wdt, kind="ExternalInput")
    whhT = nc.dram_tensor("whhT", [E, 3 * E], wdt, kind="ExternalInput")
    we2dT = nc.dram_tensor("we2dT", [E, V], wdt, kind="ExternalInput")
    # b_rz: cols [0:2*KE) = (b_ih+b_hh) for r,z; cols [2*KE:3*KE) = negated z part
    brz = nc.dram_tensor("brz", [128, 3 * KE], F32, kind="ExternalInput")
    bihn = nc.dram_tensor("bihn", [128, KE], F32, kind="ExternalInput")
    bhhn = nc.dram_tensor("bhhn", [128, KE], F32, kind="ExternalInput")
    be2d = nc.dram_tensor("be2d", [128, KV], F32, kind="ExternalInput")
    # unnormalized exp(logits) + per-step 1/rowsum; the host normalizes
    # during the gather (identical arithmetic to an on-device multiply,
    # since the on-device product would read the same rounded exp tiles)
    edt = BF16 if (MM_DT == BF16 and not OUT_F32) else F32
    out_e = nc.dram_tensor("out_e", [T, V, B], edt, kind="ExternalOutput")
    out_r = nc.dram_tensor("out_r", [T, 1, B], F32, kind="ExternalOutput")

    with tile.TileContext(nc) as tc:
        with (
            tc.tile_pool(name="w", bufs=1) as wp,
            tc.tile_pool(name="sb", bufs=1) as sb,
            tc.tile_pool(name="ps", bufs=1, space="PSUM") as pp,
        ):
            # ---- persistent weights, in first-use order (w_hh feeds t=0) ----
            def load_w(name, dram_ap, cols):
                if MM_DT == BF16:
                    wt = wp.tile([128, cols], BF16, name=name, tag=name)
                    nc.sync.dma_start(wt[:], dram_ap)
                else:
                    st = sb.tile([128, cols], F32, name="stage", tag="stage", bufs=2)
                    nc.sync.dma_start(st[:], dram_ap)
                    wt = wp.tile([128, cols], F32R, name=name, tag=name)
                    nc.vector.tensor_copy(wt[:], st[:])
                return wt

            # initial state h = x first (the first gh matmul needs it)
            hT = []  # fp32 master
            hM = []  # MM_DT matmul copy
            for m in range(KE):
                hf = sb.tile([128, B], F32, name="h", tag="h", bufs=8)
                nc.sync.dma_start(hf[:], xT[m * 128 : (m + 1) * 128, :])
                hT.append(hf)
                hm = sb.tile([128, B], MM_DT, name="hmm", tag="hmm", bufs=8)
                nc.scalar.copy(hm[:], hf[:])
                hM.append(hm)

            w_hh = [
                load_w(f"w_hh{k}", whhT[k * 128 : (k + 1) * 128, :], 3 * E)
                for k in range(KE)
            ]
            w_e2d = [
                load_w(f"w_e2d{k}", we2dT[k * 128 : (k + 1) * 128, :], V)
                for k in range(KE)
            ]
            w_d2e = [
                load_w(f"w_d2e{k}", wd2eT[k * 128 : (k + 1) * 128, :], E)
                for k in range(KV)
            ]
            w_ih = [
                load_w(f"w_ih{k}", wihT[k * 128 : (k + 1) * 128, :], 3 * E)
                for k in range(KE)
            ]

            b_rz = wp.tile([128, 3 * KE], F32, name="b_rz", tag="b_rz")
            nc.sync.dma_start(b_rz[:], brz[:])
            b_e2d = wp.tile([128, KV], F32, name="b_e2d", tag="b_e2d")
            nc.sync.dma_start(b_e2d[:], be2d[:])
            b_ihn = wp.tile([128, KE], F32, name="b_ihn", tag="b_ihn")
            nc.sync.dma_start(b_ihn[:], bihn[:])
            b_hhn = wp.tile([128, KE], F32, name="b_hhn", tag="b_hhn")
            nc.sync.dma_start(b_hhn[:], bhhn[:])

            ones_f32 = wp.tile([128, 128], F32, name="ones_f32", tag="ones_f32")
            nc.gpsimd.memset(ones_f32[:], 1.0)
            ones_mat = wp.tile([128, 128], MM_DT, name="ones_mat", tag="ones_mat")
            nc.vector.tensor_copy(ones_mat[:], ones_f32[:])

            eT = None  # unnormalized exp(logits) of previous step (MM_DT)
            rbc = None  # 1/rowsum broadcast [128, B] fp32

            for t in range(T):
                # ---- emb = softmax_{t-1} @ W_d2e.T (feature-major [E, B]);
                # normalization folded into the PSUM drain ----
                embT = None
                if t > 0:
                    H = B // 2
                    embT = []
                    for m in range(KE):
                        ps = pp.tile([128, B], F32, name="ps_mm", tag="mm", bufs=7)
                        for k in range(KV):
                            nc.tensor.matmul(
                                ps[:],
                                w_d2e[k][:, m * 128 : (m + 1) * 128],
                                eT[k][:],
                                start=(k == 0),
                                stop=(k == KV - 1),
                            )
                        ev = sb.tile([128, B], MM_DT, name="embT", tag="embT", bufs=8)
                        nc.vector.tensor_mul(ev[:], ps[:], rbc[:])
                        embT.append(ev)

                # ---- gates r, z: sigmoid(gh + gx + biases); gh emitted first
                # so the PE has emb-independent work during the softmax tail.
                # z additionally drains (1-z) via sigmoid(-x) and z*h early ----
                r_g = []
                z_g = []  # z * h_old
                omz_g = []  # 1 - z
                for g in range(2):
                    for m in range(KE):
                        col = g * E + m * 128
                        ps = pp.tile([128, B], F32, name="ps_mm", tag="mm", bufs=7)
                        for k in range(KE):
                            nc.tensor.matmul(
                                ps[:],
                                w_hh[k][:, col : col + 128],
                                hM[k][:],
                                start=(k == 0),
                                stop=(t == 0 and k == KE - 1),
                            )
                        if t > 0:
                            for k in range(KE):
                                nc.tensor.matmul(
                                    ps[:],
                                    w_ih[k][:, col : col + 128],
                                    embT[k][:],
                                    start=False,
                                    stop=(k == KE - 1),
                                )
                        j = g * KE + m
                        if g == 0:
                            gt = sb.tile(
                                [128, B], F32, name="gate_r", tag="gate_r", bufs=4
                            )
                            nc.scalar.activation(
                                gt[:], ps[:], AF.Sigmoid, bias=b_rz[:, j : j + 1]
                            )
                            r_g.append(gt)
                        else:
                            zt = sb.tile(
                                [128, B], F32, name="gate_z", tag="gate_z", bufs=4
                            )
                            nc.scalar.activation(
                                zt[:], ps[:], AF.Sigmoid, bias=b_rz[:, j : j + 1]
                            )
                            oz = sb.tile(
                                [128, B], F32, name="gate_omz", tag="gate_omz", bufs=4
                            )
                            nj = 2 * KE + m
                            nc.scalar.activation(
                                oz[:],
                                ps[:],
                                AF.Sigmoid,
                                bias=b_rz[:, nj : nj + 1],
                                scale=-1.0,
                            )
                            omz_g.append(oz)
                            zh = sb.tile(
                                [128, B], F32, name="zh", tag="zh", bufs=4
                            )
                            nc.gpsimd.tensor_mul(zh[:], zt[:], hT[m][:])
                            z_g.append(zh)

                # ---- n gate: tanh(xn + b_ihn + r * (hn + b_hhn)); b_hhn lands
                # via an off-critical-path Identity drain, b_ihn rides the
                # Tanh's per-partition bias ----
                hnb_g = []
                for m in range(KE):
                    col = 2 * E + m * 128
                    ps = pp.tile([128, B], F32, name="ps_mm", tag="mm", bufs=7)
                    for k in range(KE):
                        nc.tensor.matmul(
                            ps[:],
                            w_hh[k][:, col : col + 128],
                            hM[k][:],
                            start=(k == 0),
                            stop=(k == KE - 1),
                        )
                    hv = sb.tile([128, B], F32, name="hnb", tag="hnb", bufs=4)
                    nc.scalar.activation(
                        hv[:], ps[:], AF.Identity, bias=b_hhn[:, m : m + 1]
                    )
                    hnb_g.append(hv)

                ps_xn = None
                if t > 0:
                    ps_xn = []
                    for m in range(KE):
                        col = 2 * E + m * 128
                        ps = pp.tile([128, B], F32, name="ps_mm", tag="mm", bufs=7)
                        for k in range(KE):
                            nc.tensor.matmul(
                                ps[:],
                                w_ih[k][:, col : col + 128],
                                embT[k][:],
                                start=(k == 0),
                                stop=(k == KE - 1),
                            )
                        ps_xn.append(ps)

                # ---- h' = (1-z)*n + z*h, bf16 matmul copy written first ----
                hN = []
                hNM = []
                for m in range(KE):
                    t2 = sb.tile([128, B], F32, name="t2", tag="t2", bufs=4)
                    nc.vector.tensor_mul(t2[:], r_g[m][:], hnb_g[m][:])
                    if t > 0:
                        nc.vector.tensor_add(t2[:], t2[:], ps_xn[m][:])
                    nc.scalar.activation(
                        t2[:], t2[:], AF.Tanh, bias=b_ihn[:, m : m + 1]
                    )  # n, in place
                    nc.vector.tensor_mul(t2[:], t2[:], omz_g[m][:])  # (1-z)*n
                    hm = sb.tile([128, B], MM_DT, name="hmm", tag="hmm", bufs=8)
                    nc.vector.tensor_add(hm[:], t2[:], z_g[m][:])
                    hNM.append(hm)
                    hf = sb.tile([128, B], F32, name="h", tag="h", bufs=8)
                    nc.vector.tensor_add(hf[:], t2[:], z_g[m][:])
                    hN.append(hf)
                hT = hN
                hM = hNM

                # ---- logits = h' @ W_e2d.T + b_e2d; eT = exp(logits); the
                # row-sum matmuls (reduction over V partitions, pre-broadcast
                # via a ones matrix) are interleaved so the reciprocal can
                # start as soon as the last exp lands ----
                eT = []
                ps_s = pp.tile([128, B], F32, name="ps_s", tag="srow", bufs=1)
                for j in range(KV):
                    ps = pp.tile([128, B], F32, name="ps_mm", tag="mm", bufs=7)
                    for k in range(KE):
                        nc.tensor.matmul(
                            ps[:],
                            w_e2d[k][:, j * 128 : (j + 1) * 128],
                            hM[k][:],
                            start=(k == 0),
                            stop=(k == KE - 1),
                        )
                    if OUT_F32 and MM_DT == BF16:
                        # fp32 exp for the output stream; rounded copy for
                        # the PE (removes the bf16 rounding from the output)
                        ef = sb.tile([128, B], F32, name="eTf", tag="eTf", bufs=4)
                        nc.scalar.activation(
                            ef[:], ps[:], AF.Exp, bias=b_e2d[:, j : j + 1]
                        )
                        nc.sync.dma_start(
                            out_e[t, j * 128 : (j + 1) * 128, :], ef[:]
                        )
                        ev = sb.tile([128, B], MM_DT, name="eT", tag="eT", bufs=12)
                        nc.vector.tensor_copy(ev[:], ef[:])
                    else:
                        ev = sb.tile([128, B], MM_DT, name="eT", tag="eT", bufs=12)
                        nc.scalar.activation(
                            ev[:], ps[:], AF.Exp, bias=b_e2d[:, j : j + 1]
                        )
                        if MM_DT == BF16:
                            nc.sync.dma_start(
                                out_e[t, j * 128 : (j + 1) * 128, :], ev[:]
                            )
                        else:
                            nc.sync.dma_start(
                                out_e[t, j * 128 : (j + 1) * 128, :],
                                ev[:].bitcast(F32),
                            )
                    eT.append(ev)
                for j in range(KV):
                    nc.tensor.matmul(
                        ps_s[:],
                        ones_mat[:],
                        eT[j][:],
                        start=(j == 0),
                        stop=(j == KV - 1),
                    )
                rbc = sb.tile([128, B], F32, name="rbc", tag="rbc", bufs=2)
                nc.vector.reciprocal(rbc[:], ps_s[:])
                nc.sync.dma_start(out_r[t, :, :], rbc[0:1, :])

    nc.compile()
    return nc


def _prep_inputs(x, W_d2e, W_ih, W_hh, b_ih, b_hh, W_e2d, b_e2d):
    E = x.shape[1]
    V = np.asarray(W_e2d).shape[0]
    KE = E // 128
    KV = V // 128

    if MM_DT == BF16:
        import ml_dtypes

        wnp = ml_dtypes.bfloat16
    else:
        wnp = np.float32

    def c(a, dt=np.float32):
        return np.ascontiguousarray(np.asarray(a, dtype=np.float32).astype(dt))

    b_ih = np.asarray(b_ih, dtype=np.float32)
    b_hh = np.asarray(b_hh, dtype=np.float32)
    brz_sum = (b_ih + b_hh)[: 2 * E].reshape(2 * KE, 128).T  # [128, 8]
    brz_negz = -(b_ih + b_hh)[E : 2 * E].reshape(KE, 128).T  # [128, 4]

    shared = {
        "wd2eT": c(np.asarray(W_d2e).T, wnp),  # [V, E]
        "wihT": c(np.asarray(W_ih).T, wnp),  # [E, 3E]
        "whhT": c(np.asarray(W_hh).T, wnp),
        "we2dT": c(np.asarray(W_e2d).T, wnp),  # [E, V]
        "brz": c(np.concatenate([brz_sum, brz_negz], axis=1)),  # [128, 12]
        "bihn": c(b_ih[2 * E :].reshape(KE, 128).T),
        "bhhn": c(b_hh[2 * E :].reshape(KE, 128).T),
        "be2d": c(np.asarray(b_e2d).reshape(KV, 128).T),
    }
    N = x.shape[0]
    B = N // N_CORES
    in_maps = []
    for core in range(N_CORES):
        m = dict(shared)
        m["xT"] = c(np.asarray(x)[core * B : (core + 1) * B, :].T)  # [E, B]
        in_maps.append(m)
    return in_maps, B


def _run(inputs, trace=False):
    from concourse.bass_utils import run_bass_kernel_spmd

    x = np.asarray(inputs["x"], dtype=np.float32)
    T = int(inputs["max_len"])
    N, E = x.shape
    V = np.asarray(inputs["W_e2d"]).shape[0]
    assert N % N_CORES == 0 and E % 128 == 0 and V % 128 == 0

    in_maps, B = _prep_inputs(
        x,
        inputs["W_d2e"],
        inputs["W_ih"],
        inputs["W_hh"],
        inputs["b_ih"],
        inputs["b_hh"],
        inputs["W_e2d"],
        inputs["b_e2d"],
    )
    nc = _build(T, B, E, V)
    res = run_bass_kernel_spmd(
        nc, in_maps, core_ids=list(range(N_CORES)), trace=trace
    )

    full = np.empty((T, N, V), dtype=np.float32)
    for core in range(N_CORES):
        e = np.asarray(res.results[core]["out_e"], dtype=np.float32)  # [T, V, B]
        rinv = np.asarray(res.results[core]["out_r"], dtype=np.float32)  # [T, 1, B]
        full[:, core * B : (core + 1) * B, :] = np.transpose(e * rinv, (0, 2, 1))
    return full, res


def kernel(**inputs):
    full, _ = _run(inputs, trace=False)
    return full


def run_traced(**inputs):
    return _run(inputs, trace=True)


# revision 42
# speedup vs baseline: 1.0492x; 1.0006x over previous
"""Trainium2 Bass kernel for a differentiable GRU decoder.

Per step t (max_len=32 steps), batch N=4096, E=512, V=1024:
    emb    = probs_{t-1} @ W_d2e.T            # [N, E]
    h      = GRUCell(emb, h)                  # [N, E]
    logits = h @ W_e2d.T + b_e2d              # [N, V]
    probs  = softmax(logits)                  # [N, V]  -> output[t]

Sharding: data-parallel over N across 8 cores (512 rows each), weights
replicated, the 32-step scan stays local per core — no collectives.

Design notes:
- Feature-major on-chip layout ([features on partitions, batch on free])
  lets every matmul chain without transposes; the per-core output is
  written feature-major as [T, V, 512] and un-transposed on the host
  during the gather.
- Matmul operands stream as bf16 by default (DEC_MM_DT=f32r switches to
  the fp32r path: ~2x slower, ~2e-4 output error vs ~4e-3 for bf16).
  PSUM accumulation is fp32 either way.  The GRU state keeps an fp32
  master for the elementwise update plus a rounded copy for the PE.
- Softmax row sums (a reduction over partitions) are computed with a
  ones-MATRIX matmul whose output lands pre-broadcast on all 128
  partitions (matmul cost scales only with the free dim, so M=128 costs
  the same as M=1); the reciprocal runs on the vector engine off the
  PE's critical path.  exp(logits) stays unnormalized: the 1/sum scale
  folds into the PSUM drain of the next step's emb matmul (per-batch
  scaling commutes with the contraction), and the OUTPUT normalize
  happens on the host during the gather — arithmetic identical to an
  on-device multiply, since that multiply would read the same rounded
  exp tiles; the device streams out exp(logits) plus one 1/sum row per
  step (DEC_OUT=f32 keeps an fp32 exp stream for ~2x lower error at
  ~5% more time).
- The recurrence's serial tail (last gate matmul -> h' -> logits) is
  kept short: h' = (1-z)*n + z*h with (1-z) from a second sigmoid drain
  (scale=-1, negated bias), z*h computed early off-path on the idle
  GPSIMD engine, b_ihn riding the Tanh's per-partition bias, and the
  bf16 copy of h' that feeds the logits matmuls written before the
  fp32 master.
- In each gate's PSUM accumulation the recurrent (W_hh @ h) half is
  emitted before the (W_ih @ emb) half, giving the scheduler
  emb-independent matmuls to run while the softmax reciprocal resolves.
"""

import os
import sys
import types

import numpy as np

import concourse.bacc as bacc
import concourse.mybir as mybir
import concourse.tile as tile

F32 = mybir.dt.float32
F32R = mybir.dt.float32r
BF16 = mybir.dt.bfloat16
AF = mybir.ActivationFunctionType

N_CORES = 8
MM_DT = F32R if os.environ.get("DEC_MM_DT", "bf16") == "f32r" else BF16
OUT_F32 = os.environ.get("DEC_OUT", "bf16") == "f32"


def _install_ntff_hook():
    """Register the axon NTFF profiling hook if the image's antenv lacks it."""
    try:
        import antenv.axon_hooks  # noqa: F401
        return
    except ImportError:
        pass
    try:
        from trn_agent_boot.trn_boot import _ntff_profile_via_ctypes

        hook = _ntff_profile_via_ctypes("/opt/axon/libaxon_pjrt.so")
    except Exception:
        hook = None
    mod = types.ModuleType("antenv.axon_hooks")
    mod.get_axon_ntff_profile_hook = lambda: hook
    mod.set_axon_ntff_profile_hook = lambda h: None
    sys.modules["antenv.axon_hooks"] = mod


_install_ntff_hook()


def _build(T, B, E, V):
    """Build the per-core Bacc module. B = per-core batch (free dim)."""
    KE = E // 128  # E-tiles (4)
    KV = V // 128  # V-tiles (8)

    nc = bacc.Bacc(None, target_bir_lowering=False)

    wdt = F32 if MM_DT == F32R else BF16  # dram dtype for weight streams
    xT = nc.dram_tensor("xT", [E, B], F32, kind="ExternalInput")
    wd2eT = nc.dram_tensor("wd2eT", [V, E], wdt, kind="ExternalInput")
    wihT = nc.dram_tensor("wihT", [E, 3 * E], wdt, kind="ExternalInput")
    whhT = nc.dram_tensor("whhT", [E, 3 * E], wdt, kind="ExternalInput")
    we2dT = nc.dram_tensor("we2dT", [E, V], wdt, kind="ExternalInput")
    # b_rz: cols [0:2*KE) = (b_ih+b_hh) for r,z; cols [2*KE:3*KE) = negated z part
    brz = nc.dram_tensor("brz", [128, 3 * KE], F32, kind="ExternalInput")
    bihn = nc.dram_tensor("bihn", [128, KE], F32, kind="ExternalInput")
    bhhn = nc.dram_tensor("bhhn", [128, KE], F32, kind="ExternalInput")
    be2d = nc.dram_tensor("be2d", [128, KV], F32, kind="ExternalInput")
    # unnormalized exp(logits) + per-step 1/rowsum; the host normalizes
    # during the gather (identical arithmetic to an on-device multiply,
    # since the on-device product would read the same rounded exp tiles)
    edt = BF16 if (MM_DT == BF16 and not OUT_F32) else F32
    out_e = nc.dram_tensor("out_e", [T, V, B], edt, kind="ExternalOutput")
    out_r = nc.dram_tensor("out_r", [T, 1, B], F32, kind="ExternalOutput")

    with tile.TileContext(nc) as tc:
        with (
            tc.tile_pool(name="w", bufs=1) as wp,
            tc.tile_pool(name="sb", bufs=1) as sb,
            tc.tile_pool(name="ps", bufs=1, space="PSUM") as pp,
        ):
            # ---- persistent weights, in first-use order (w_hh feeds t=0) ----
            def load_w(name, dram_ap, cols):
                if MM_DT == BF16:
                    wt = wp.tile([128, cols], BF16, name=name, tag=name)
                    nc.sync.dma_start(wt[:], dram_ap)
                else:
                    st = sb.tile([128, cols], F32, name="stage", tag="stage", bufs=2)
                    nc.sync.dma_start(st[:], dram_ap)
                    wt = wp.tile([128, cols], F32R, name=name, tag=name)
                    nc.vector.tensor_copy(wt[:], st[:])
                return wt

            # initial state h = x first (the first gh matmul needs it)
            hT = []  # fp32 master
            hM = []  # MM_DT matmul copy
            for m in range(KE):
                hf = sb.tile([128, B], F32, name="h", tag="h", bufs=8)
                nc.sync.dma_start(hf[:], xT[m * 128 : (m + 1) * 128, :])
                hT.append(hf)
                hm = sb.tile([128, B], MM_DT, name="hmm", tag="hmm", bufs=8)
                nc.scalar.copy(hm[:], hf[:])
                hM.append(hm)

            w_hh = [
                load_w(f"w_hh{k}", whhT[k * 128 : (k + 1) * 128, :], 3 * E)
                for k in range(KE)
            ]
            w_e2d = [
                load_w(f"w_e2d{k}", we2dT[k * 128 : (k + 1) * 128, :], V)
                for k in range(KE)
            ]
            w_d2e = [
                load_w(f"w_d2e{k}", wd2eT[k * 128 : (k + 1) * 128, :], E)
                for k in range(KV)
            ]
            w_ih = [
                load_w(f"w_ih{k}", wihT[k * 128 : (k + 1) * 128, :], 3 * E)
                for k in range(KE)
            ]

            b_rz = wp.tile([128, 3 * KE], F32, name="b_rz", tag="b_rz")
            nc.sync.dma_start(b_rz[:], brz[:])
            b_e2d = wp.tile([128, KV], F32, name="b_e2d", tag="b_e2d")
            nc.sync.dma_start(b_e2d[:], be2d[:])
            b_ihn = wp.tile([128, KE], F32, name="b_ihn", tag="b_ihn")
            nc.sync.dma_start(b_ihn[:], bihn[:])
            b_hhn = wp.tile([128, KE], F32, name="b_hhn", tag="b_hhn")
            nc.sync.dma_start(b_hhn[:], bhhn[:])

            ones_f32 = wp.tile([128, 128], F32, name="ones_f32", tag="ones_f32")
            nc.gpsimd.memset(ones_f32[:], 1.0)
            ones_mat = wp.tile([128, 128], MM_DT, name="ones_mat", tag="ones_mat")
            nc.vector.tensor_copy(ones_mat[:], ones_f32[:])

            eT = None  # unnormalized exp(logits) of previous step (MM_DT)
            rbc = None  # 1/rowsum broadcast [128, B] fp32

            for t in range(T):
                # ---- emb = softmax_{t-1} @ W_d2e.T (feature-major [E, B]);
                # normalization folded into the PSUM drain ----
                embT = None
                if t > 0:
                    embT = []
                    for m in range(KE):
                        ps = pp.tile([128, B], F32, name="ps_mm", tag="mm", bufs=7)
                        for k in range(KV):
                            nc.tensor.matmul(
                                ps[:],
                                w_d2e[k][:, m * 128 : (m + 1) * 128],
                                eT[k][:],
                                start=(k == 0),
                                stop=(k == KV - 1),
                            )
                        ev = sb.tile([128, B], MM_DT, name="embT", tag="embT", bufs=8)
                        nc.vector.tensor_mul(ev[:], ps[:], rbc[:])
                        embT.append(ev)

                # ---- gates r, z: sigmoid(gh + gx + biases); gh emitted first
                # so the PE has emb-independent work during the softmax tail.
                # z additionally drains (1-z) via sigmoid(-x) and z*h early ----
                r_g = []
                z_g = []  # z * h_old
                omz_g = []  # 1 - z
                for g in range(2):
                    for m in range(KE):
                        col = g * E + m * 128
                        ps = pp.tile([128, B], F32, name="ps_mm", tag="mm", bufs=7)
                        for k in range(KE):
                            nc.tensor.matmul(
                                ps[:],
                                w_hh[k][:, col : col + 128],
                                hM[k][:],
                                start=(k == 0),
                                stop=(t == 0 and k == KE - 1),
                            )
                        if t > 0:
                            for k in range(KE):
                                nc.tensor.matmul(
                                    ps[:],
                                    w_ih[k][:, col : col + 128],
                                    embT[k][:],
                                    start=False,
                                    stop=(k == KE - 1),
                                )
                        j = g * KE + m
                        if g == 0:
                            gt = sb.tile(
                                [128, B], F32, name="gate_r", tag="gate_r", bufs=4
                            )
                            nc.scalar.activation(
                                gt[:], ps[:], AF.Sigmoid, bias=b_rz[:, j : j + 1]
                            )
                            r_g.append(gt)
                        else:
                            zt = sb.tile(
                                [128, B], F32, name="gate_z", tag="gate_z", bufs=4
                            )
                            nc.scalar.activation(
                                zt[:], ps[:], AF.Sigmoid, bias=b_rz[:, j : j + 1]
                            )
                            oz = sb.tile(
                                [128, B], F32, name="gate_omz", tag="gate_omz", bufs=4
                            )
                            nj = 2 * KE + m
                            nc.scalar.activation(
                                oz[:],
                                ps[:],
                                AF.Sigmoid,
                                bias=b_rz[:, nj : nj + 1],
                                scale=-1.0,
                            )
                            omz_g.append(oz)
                            zh = sb.tile(
                                [128, B], F32, name="zh", tag="zh", bufs=4
                            )
                            nc.gpsimd.tensor_mul(zh[:], zt[:], hT[m][:])
                            z_g.append(zh)

                # ---- n gate: tanh(xn + b_ihn + r * (hn + b_hhn)); b_hhn lands
                # via an off-critical-path Identity drain, b_ihn rides the
                # Tanh's per-partition bias ----
                hnb_g = []
                for m in range(KE):
                    col = 2 * E + m * 128
                    ps = pp.tile([128, B], F32, name="ps_mm", tag="mm", bufs=7)
                    for k in range(KE):
                        nc.tensor.matmul(
                            ps[:],
                            w_hh[k][:, col : col + 128],
                            hM[k][:],
                            start=(k == 0),
                            stop=(k == KE - 1),
                        )
                    hv = sb.tile([128, B], F32, name="hnb", tag="hnb", bufs=4)
                    nc.scalar.activation(
                        hv[:], ps[:], AF.Identity, bias=b_hhn[:, m : m + 1]
                    )
                    hnb_g.append(hv)

                ps_xn = None
                if t > 0:
                    ps_xn = []
                    for m in range(KE):
                        col = 2 * E + m * 128
                        ps = pp.tile([128, B], F32, name="ps_mm", tag="mm", bufs=7)
                        for k in range(KE):
                            nc.tensor.matmul(
                                ps[:],
                                w_ih[k][:, col : col + 128],
                                embT[k][:],
                                start=(k == 0),
                                stop=(k == KE - 1),
                            )
                        ps_xn.append(ps)

                # ---- h' = (1-z)*n + z*h, bf16 matmul copy written first ----
                hN = []
                hNM = []
                for m in range(KE):
                    t2 = sb.tile([128, B], F32, name="t2", tag="t2", bufs=4)
                    nc.vector.tensor_mul(t2[:], r_g[m][:], hnb_g[m][:])
                    if t > 0:
                        nc.vector.tensor_add(t2[:], t2[:], ps_xn[m][:])
                    nc.scalar.activation(
                        t2[:], t2[:], AF.Tanh, bias=b_ihn[:, m : m + 1]
                    )  # n, in place
                    nc.vector.tensor_mul(t2[:], t2[:], omz_g[m][:])  # (1-z)*n
                    hm = sb.tile([128, B], MM_DT, name="hmm", tag="hmm", bufs=8)
                    nc.vector.tensor_add(hm[:], t2[:], z_g[m][:])
                    hNM.append(hm)
                    hf = sb.tile([128, B], F32, name="h", tag="h", bufs=8)
                    nc.vector.tensor_add(hf[:], t2[:], z_g[m][:])
                    hN.append(hf)
                hT = hN
                hM = hNM

                # ---- logits = h' @ W_e2d.T + b_e2d; eT = exp(logits); the
                # row-sum matmuls (reduction over V partitions, pre-broadcast
                # via a ones matrix) are interleaved so the reciprocal can
                # start as soon as the last exp lands ----
                eT = []
                ps_s = pp.tile([128, B], F32, name="ps_s", tag="srow", bufs=1)
                for j in range(KV):
                    ps = pp.tile([128, B], F32, name="ps_mm", tag="mm", bufs=7)
                    for k in range(KE):
                        nc.tensor.matmul(
                            ps[:],
                            w_e2d[k][:, j * 128 : (j + 1) * 128],
                            hM[k][:],
                            start=(k == 0),
                            stop=(k == KE - 1),
                        )
                    if OUT_F32 and MM_DT == BF16:
                        # fp32 exp for the output stream; rounded copy for
                        # the PE (removes the bf16 rounding from the output)
                        ef = sb.tile([128, B], F32, name="eTf", tag="eTf", bufs=4)
                        nc.scalar.activation(
                            ef[:], ps[:], AF.Exp, bias=b_e2d[:, j : j + 1]
                        )
                        nc.sync.dma_start(
                            out_e[t, j * 128 : (j + 1) * 128, :], ef[:]
                        )
                        ev = sb.tile([128, B], MM_DT, name="eT", tag="eT", bufs=12)
                        nc.vector.tensor_copy(ev[:], ef[:])
                    else:
                        ev = sb.tile([128, B], MM_DT, name="eT", tag="eT", bufs=12)
                        nc.scalar.activation(
                            ev[:], ps[:], AF.Exp, bias=b_e2d[:, j : j + 1]
                        )
                        if MM_DT == BF16:
                            nc.sync.dma_start(
                                out_e[t, j * 128 : (j + 1) * 128, :], ev[:]
                            )
                        else:
                            nc.sync.dma_start(
                                out_e[t, j * 128 : (j + 1) * 128, :],
                                ev[:].bitcast(F32),
                            )
                    eT.append(ev)
                for j in range(KV):
                    nc.tensor.matmul(
                        ps_s[:],
                        ones_mat[:],
                        eT[j][:],
                        start=(j == 0),
                        stop=(j == KV - 1),
                    )
                rbc = sb.tile([128, B], F32, name="rbc", tag="rbc", bufs=2)
                nc.vector.reciprocal(rbc[:], ps_s[:])
                nc.sync.dma_start(out_r[t, :, :], rbc[0:1, :])

    nc.compile()
    return nc


def _prep_inputs(x, W_d2e, W_ih, W_hh, b_ih, b_hh, W_e2d, b_e2d):
    E = x.shape[1]
    V = np.asarray(W_e2d).shape[0]
    KE = E // 128
    KV = V // 128

    if MM_DT == BF16:
        import ml_dtypes

        wnp = ml_dtypes.bfloat16
    else:
        wnp = np.float32

    def c(a, dt=np.float32):
        return np.ascontiguousarray(np.asarray(a, dtype=np.float32).astype(dt))

    b_ih = np.asarray(b_ih, dtype=np.float32)
    b_hh = np.asarray(b_hh, dtype=np.float32)
    brz_sum = (b_ih + b_hh)[: 2 * E].reshape(2 * KE, 128).T  # [128, 8]
    brz_negz = -(b_ih + b_hh)[E : 2 * E].reshape(KE, 128).T  # [128, 4]

    shared = {
        "wd2eT": c(np.asarray(W_d2e).T, wnp),  # [V, E]
        "wihT": c(np.asarray(W_ih).T, wnp),  # [E, 3E]
        "whhT": c(np.asarray(W_hh).T, wnp),
        "we2dT": c(np.asarray(W_e2d).T, wnp),  # [E, V]
        "brz": c(np.concatenate([brz_sum, brz_negz], axis=1)),  # [128, 12]
        "bihn": c(b_ih[2 * E :].reshape(KE, 128).T),
        "bhhn": c(b_hh[2 * E :].reshape(KE, 128).T),
        "be2d": c(np.asarray(b_e2d).reshape(KV, 128).T),
    }
    N = x.shape[0]
    B = N // N_CORES
    in_maps = []
    for core in range(N_CORES):
        m = dict(shared)
        m["xT"] = c(np.asarray(x)[core * B : (core + 1) * B, :].T)  # [E, B]
        in_maps.append(m)
    return in_maps, B


def _run(inputs, trace=False):
    from concourse.bass_utils import run_bass_kernel_spmd

    x = np.asarray(inputs["x"], dtype=np.float32)
    T = int(inputs["max_len"])
    N, E = x.shape
    V = np.asarray(inputs["W_e2d"]).shape[0]
    assert N % N_CORES == 0 and E % 128 == 0 and V % 128 == 0

    in_maps, B = _prep_inputs(
        x,
        inputs["W_d2e"],
        inputs["W_ih"],
        inputs["W_hh"],
        inputs["b_ih"],
        inputs["b_hh"],
        inputs["W_e2d"],
        inputs["b_e2d"],
    )
    nc = _build(T, B, E, V)
    res = run_bass_kernel_spmd(
        nc, in_maps, core_ids=list(range(N_CORES)), trace=trace
    )

    full = np.empty((T, N, V), dtype=np.float32)
    for core in range(N_CORES):
        e = np.asarray(res.results[core]["out_e"], dtype=np.float32)  # [T, V, B]
        rinv = np.asarray(res.results[core]["out_r"], dtype=np.float32)  # [T, 1, B]
        full[:, core * B : (core + 1) * B, :] = np.transpose(e * rinv, (0, 2, 1))
    return full, res


def kernel(**inputs):
    full, _ = _run(inputs, trace=False)
    return full


def run_traced(**inputs):
    return _run(inputs, trace=True)
